# revision 1
# baseline (speedup 1.0000x reference)
"""Causal self-attention Bass kernel for 8 TRN2 NeuronCores.

Problem: B=4, T=2048, C=1024, H=16 heads, head_dim=64, fp32.
    q = x @ Wq.T ; k = x @ Wk.T ; v = x @ Wv.T          (per head)
    att = softmax(mask(q k^T / 8))
    y = att @ v ; out = y @ Wp.T

Sharding (8 cores): 4-way data parallel over batch x 2-way tensor
parallel over heads. Core c handles batch c//2 and heads 8*(c%2)..+8.
Wq/Wk/Wv column-parallel, Wp row-parallel; the partial outputs of the
two head-halves of each batch are summed on the host (the "all-reduce"
of row-parallel Wp).

Device dataflow (all transposed, so no on-chip transposes are needed):
    xT [C, T] (host-pretransposed) ->
    qT/kT = WqT.T-slices @ xT   [512, T]  (pairs of heads on partitions)
    v     = xT.T-tiles @ WvT    [T, 512]
    scoresT[k, q] = kT.T @ qT   (k on partitions -> softmax sum over k
                                 via a ones-column appended to v)
    expT = exp(0.125 * scoresT) (no max subtraction: scores ~ N(0, 0.4))
    yT[d, q] (+ row of sums) = v_aug.T @ expT, accumulated over k tiles
    out[t, c] = yT.T-tiles @ WpT, accumulated over local j

Projections and the output projection compute in float32r (TensorE
full rate, ~1.5e-4 rel err); the attention core (q/k/v/exp operands)
uses bf16, whose fast weight load keeps the PE's mixed-shape
instruction stream at full rate. End-to-end l2 rel err ~2.5e-3.
"""

from contextlib import ExitStack

import numpy as np

import concourse.bass as bass
import concourse.tile as tile
from concourse import bacc, mybir

F32 = mybir.dt.float32
F32R = mybir.dt.float32r
BF16 = mybir.dt.bfloat16

B, T, C, H, D = 4, 2048, 1024, 16, 64
NCORES = 8
JL = 512            # local j dims per core (8 heads * 64)
NPAIR = 4           # local head pairs
CI = C // 128       # 8 c-tiles
NT = T // 128       # 16 t/k tiles
NQC = T // 512      # 4 q chunks

_CACHED_NC = None


def build_nc():
    nc = bacc.Bacc(None)

    xT = nc.dram_tensor("xT", [C, T], F32R, kind="ExternalInput")
    wqT = nc.dram_tensor("wqT", [C, JL], F32R, kind="ExternalInput")
    wkT = nc.dram_tensor("wkT", [C, JL], F32R, kind="ExternalInput")
    wvT = nc.dram_tensor("wvT", [C, JL], F32R, kind="ExternalInput")
    wpT = nc.dram_tensor("wpT", [JL, C], F32R, kind="ExternalInput")
    out = nc.dram_tensor("out", [T, C], F32, kind="ExternalOutput")
    # bounce buffer for broadcasting softmax reciprocals across partitions
    rcd = nc.dram_tensor("rcd", [NPAIR, NQC, 2, 512], F32)

    xT_r = xT.rearrange("(ci p) t -> p ci t", p=128)
    wq_r = wqT.rearrange("(ci p) j -> p ci j", p=128)
    wk_r = wkT.rearrange("(ci p) j -> p ci j", p=128)
    wv_r = wvT.rearrange("(ci p) j -> p ci j", p=128)
    wp_r = wpT.rearrange("(ji p) c -> p ji c", p=128)

    with tile.TileContext(nc) as tc, ExitStack() as ctx:
        pm = ctx.enter_context(tc.tile_pool(name="pm", bufs=1))
        qkp = ctx.enter_context(tc.tile_pool(name="qkp", bufs=1))
        expp = ctx.enter_context(tc.tile_pool(name="expp", bufs=3))
        bcp = ctx.enter_context(tc.tile_pool(name="bcp", bufs=1))
        rcp = ctx.enter_context(tc.tile_pool(name="rcp", bufs=1))
        stp = ctx.enter_context(tc.tile_pool(name="stp", bufs=4))
        gp = ctx.enter_context(tc.tile_pool(name="gp", bufs=2, space="PSUM"))
        yp = ctx.enter_context(tc.tile_pool(name="yp", bufs=4, space="PSUM"))

        # v with a ones column prepended per head (so the softmax sums land
        # on psum partition 0, where reciprocal_approx_fast works), plus 64
        # pad columns so every per-head lhsT can be read as [128, 128] --
        # NumWeights==128 enables the fast weight load path.
        VW = D + 1
        v_sb = pm.tile([128, NT, 8 * VW + 64], BF16)
        v_view = v_sb[:, :, 0 : 8 * VW].rearrange("p n (h w) -> p n h w", w=VW)
        # (causal triangles are zeroed post-exp via gpsimd.affine_select)
        ones_col = pm.tile([128, NT, 8, 1], F32)
        nc.vector.memset(ones_col[:], 1.0)
        nc.vector.tensor_copy(v_view[:, :, :, 0:1], ones_col[:])
        nc.vector.memset(v_sb[:, :, 8 * VW : 8 * VW + 64], 0.0)

        qT_all = qkp.tile([128, NPAIR, T], BF16, tag="qT_all")
        kT_all = qkp.tile([128, NPAIR, T], BF16, tag="kT_all")

        # ---- phase 1: projections (stream xT by t-chunks) -----------------
        with (
            tc.tile_pool(name="ph1w", bufs=1) as wpool,
            tc.tile_pool(name="ph1x", bufs=2) as xpool,
        ):
            wq_sb = wpool.tile([128, CI, JL], F32R, tag="wq")
            wk_sb = wpool.tile([128, CI, JL], F32R, tag="wk")
            wv_sb = wpool.tile([128, CI, JL], F32R, tag="wv")
            # split weight/x DMAs per c-tile so the first matmuls can start
            # as soon as the first slices land; queue order matters (FIFO per
            # ring), so interleave chunk-0 x right after wq
            for ci in range(CI):
                nc.sync.dma_start(wq_sb[:, ci, :], wq_r[:, ci, :])
            xt0 = xpool.tile([128, CI, 512], F32R, tag="xt")
            for ci in range(CI):
                nc.sync.dma_start(xt0[:, ci, :], xT_r[:, ci, 0:512])
            for ci in range(CI):
                nc.sync.dma_start(wk_sb[:, ci, :], wk_r[:, ci, :])
            for ci in range(CI):
                nc.sync.dma_start(wv_sb[:, ci, :], wv_r[:, ci, :])

            for tch in range(NQC):
                ts_ = slice(tch * 512, tch * 512 + 512)
                if tch == 0:
                    xt = xt0
                else:
                    xt = xpool.tile([128, CI, 512], F32R, tag="xt")
                    for ci in range(CI):
                        nc.sync.dma_start(xt[:, ci, :], xT_r[:, ci, ts_])

                for w_sb, dst in ((wq_sb, qT_all), (wk_sb, kT_all)):
                    for pr in range(NPAIR):
                        acc = gp.tile([128, 2, 512], F32, tag="g")
                        for ci in range(CI):
                            nc.tensor.matmul(
                                acc[:, 0, :],
                                w_sb[:, ci, pr * 128 : pr * 128 + 128],
                                xt[:, ci, :],
                                start=(ci == 0),
                                stop=(ci == CI - 1),
                            )
                        nc.vector.tensor_copy(dst[:, pr, ts_], acc[:, 0, :])

                for tl in range(4):
                    ti = tch * 4 + tl
                    acc = gp.tile([128, 2, 512], F32, tag="g")
                    for ci in range(CI):
                        nc.tensor.matmul(
                            acc[:, 0, :],
                            xt[:, ci, tl * 128 : tl * 128 + 128],
                            wv_sb[:, ci, :],
                            start=(ci == 0),
                            stop=(ci == CI - 1),
                        )
                    nc.vector.tensor_copy(
                        v_view[:, ti, :, 1 : D + 1],
                        acc[:, 0, :].rearrange("p (h d) -> p h d", d=D),
                    )

        # ---- phase 2: attention + output projection ----------------------
        with (
            tc.tile_pool(name="ph2", bufs=1) as p2,
            tc.tile_pool(name="outp", bufs=3) as outp,
        ):
            wp_sb = p2.tile([128, NPAIR, C], F32R, tag="wp")
            nc.sync.dma_start(wp_sb[:], wp_r[:])
            yT_all = p2.tile([128, NPAIR, T], F32R, tag="yT")

            for pr in range(NPAIR):
                qlo = qT_all[0:64, pr, :]
                qhi = qT_all[64:128, pr, :]
                klo = kT_all[0:64, pr, :]
                khi = kT_all[64:128, pr, :]
                for qc in range(NQC):
                    nkt = 4 * qc + 4
                    qs = slice(qc * 512, qc * 512 + 512)
                    yA = yp.tile([128, 512], F32, tag="y")
                    yB = yp.tile([128, 512], F32, tag="y")

                    # software pipeline: issue scores/exp for kt before the PV
                    # matmuls of kt-1, so the PE never waits on ACT's exp.
                    # lhsT is [128, 128] (head's ones+v then pad/next-head
                    # cols); psum rows 65..127 are don't-care junk.
                    def emit_pv(kt, e, nkt=nkt):
                        dt = kt - 4 * qc
                        lo = dt * 128 if dt > 0 else 0
                        nc.tensor.matmul(
                            yA[:, lo:512],
                            v_sb[:, kt, 2 * pr * VW : 2 * pr * VW + 128],
                            e[:, 0, lo:512],
                            start=(kt == 0),
                            stop=(kt == nkt - 1),
                        )
                        nc.tensor.matmul(
                            yB[:, lo:512],
                            v_sb[:, kt, (2 * pr + 1) * VW : (2 * pr + 1) * VW + 128],
                            e[:, 1, lo:512],
                            start=(kt == 0),
                            stop=(kt == nkt - 1),
                        )

                    prev = None
                    for kt in range(nkt):
                        dt = kt - 4 * qc
                        ks = slice(kt * 128, kt * 128 + 128)
                        g = gp.tile([128, 2, 512], F32, tag="g")
                        nc.tensor.matmul(
                            g[:, 0, :], klo[:, ks], qlo[:, qs], start=True, stop=True
                        )
                        nc.tensor.matmul(
                            g[:, 1, :], khi[:, ks], qhi[:, qs], start=True, stop=True
                        )
                        e = expp.tile([128, 2, 512], BF16, tag="e")
                        xlo = dt * 128 if dt > 0 else 0
                        nc.scalar.activation(
                            e[:, :, xlo:512],
                            g[:, :, xlo:512],
                            mybir.ActivationFunctionType.Exp,
                            scale=0.125,
                        )
                        if dt >= 0:
                            # zero the causal triangle (k > q) of the diagonal
                            # block, on the otherwise-idle gpsimd engine
                            bs = slice(dt * 128, dt * 128 + 128)
                            for h in (0, 1):
                                nc.gpsimd.affine_select(
                                    out=e[:, h, bs],
                                    in_=e[:, h, bs],
                                    compare_op=mybir.AluOpType.is_ge,
                                    fill=0.0,
                                    base=0,
                                    pattern=[[1, 128]],
                                    channel_multiplier=-1,
                                )
                        if prev is not None:
                            emit_pv(*prev)
                        prev = (kt, e)
                    emit_pv(*prev)
                    # normalize: y / rowsum (sums live in row 0 = partition 0)
                    rc = rcp.tile([1, 2, 512], F32, tag="rc")
                    nc.vector.reciprocal_approx_fast(rc[0:1, 0, :], yA[0:1, :])
                    nc.vector.reciprocal_approx_fast(rc[0:1, 1, :], yB[0:1, :])
                    bc = bcp.tile([D + 1, 2, 512], F32, tag="bc")
                    for h in (0, 1):
                        nc.sync.dma_start(rcd[pr, qc, h : h + 1, :], rc[0:1, h, :])
                        s = rcd[pr, qc, h, :]
                        src = bass.AP(
                            tensor=s.tensor,
                            offset=s.offset,
                            ap=[[0, D + 1]] + list(s.ap),
                        )
                        nc.sync.dma_start(bc[0 : D + 1, h, :], src)
                    # y rows live on partitions 1..64; engines need 32-aligned
                    # partition bases, so multiply rows 0..64 (row 0 is the
                    # sums row scaled by its own reciprocal -- discarded) and
                    # repartition rows 1..64 into yT_all via DMA
                    stgA = stp.tile([D + 1, 512], F32R, tag="stg")
                    stgB = stp.tile([D + 1, 512], F32R, tag="stg")
                    nc.vector.tensor_mul(
                        stgA[0 : D + 1, :], yA[0 : D + 1, :], bc[0 : D + 1, 0, :]
                    )
                    nc.vector.tensor_mul(
                        stgB[0 : D + 1, :], yB[0 : D + 1, :], bc[0 : D + 1, 1, :]
                    )
                    nc.sync.dma_start(yT_all[0:64, pr, qs], stgA[1 : D + 1, :])
                    nc.sync.dma_start(yT_all[64:128, pr, qs], stgB[1 : D + 1, :])

            # output projection: out[t, c] = sum_j yT[j, t] * wpT[j, c]
            for ti in range(NT):
                tss = slice(ti * 128, ti * 128 + 128)
                for cc in range(2):
                    cs = slice(cc * 512, cc * 512 + 512)
                    acc = gp.tile([128, 2, 512], F32, tag="g")
                    for ji in range(NPAIR):
                        nc.tensor.matmul(
                            acc[:, 0, :],
                            yT_all[:, ji, tss],
                            wp_sb[:, ji, cs],
                            start=(ji == 0),
                            stop=(ji == NPAIR - 1),
                        )
                    o = outp.tile([128, 512], F32, tag="o")
                    nc.vector.tensor_copy(o[:], acc[:, 0, :])
                    nc.sync.dma_start(out[tss, cs], o[:])

    nc.finalize()
    return nc


def _get_nc():
    global _CACHED_NC
    if _CACHED_NC is None:
        _CACHED_NC = build_nc()
    return _CACHED_NC


def kernel(x, Wq, Wk, Wv, Wp):
    from concourse.bass_utils import run_bass_kernel_spmd

    x = np.asarray(x, dtype=np.float32)
    Wq = np.asarray(Wq, dtype=np.float32)
    Wk = np.asarray(Wk, dtype=np.float32)
    Wv = np.asarray(Wv, dtype=np.float32)
    Wp = np.asarray(Wp, dtype=np.float32)

    nc = _get_nc()

    xT = [np.ascontiguousarray(x[b].T) for b in range(B)]
    wqT, wkT, wvT, wpT = [], [], [], []
    for hh in range(2):
        js = slice(JL * hh, JL * hh + JL)
        wqT.append(np.ascontiguousarray(Wq[js, :].T))
        wkT.append(np.ascontiguousarray(Wk[js, :].T))
        wvT.append(np.ascontiguousarray(Wv[js, :].T))
        wpT.append(np.ascontiguousarray(Wp[:, js].T))

    in_maps = []
    for c in range(NCORES):
        b, hh = c // 2, c % 2
        in_maps.append(
            {
                "xT": xT[b],
                "wqT": wqT[hh],
                "wkT": wkT[hh],
                "wvT": wvT[hh],
                "wpT": wpT[hh],
            }
        )

    res = run_bass_kernel_spmd(nc, in_maps, core_ids=list(range(NCORES)))

    out = np.empty((B, T, C), dtype=np.float32)
    for b in range(B):
        out[b] = res.results[2 * b]["out"] + res.results[2 * b + 1]["out"]
    return out



# revision 3
# speedup vs baseline: 1.1899x; 1.1899x over previous
"""Causal self-attention Bass kernel for 8 TRN2 NeuronCores.

Problem: B=4, T=2048, C=1024, H=16 heads, head_dim=64, fp32.
    q = x @ Wq.T ; k = x @ Wk.T ; v = x @ Wv.T          (per head)
    att = softmax(mask(q k^T / 8))
    y = att @ v ; out = y @ Wp.T

Sharding (8 cores): 4-way data parallel over batch x 2-way tensor
parallel over heads. Core c handles batch c//2 and heads 8*(c%2)..+8.
Wq/Wk/Wv column-parallel, Wp row-parallel; the partial outputs of the
two head-halves of each batch are summed on the host (the "all-reduce"
of row-parallel Wp).

Device dataflow (all bf16 operands so every weight load takes the fast
path and DMA traffic halves; psum accumulation stays fp32):
    xT [C, T] (host-pretransposed) ->
    qT/kT = WqT.T-slices @ xT   [512, T]  (pairs of heads on partitions)
    v     = xT.T-tiles @ WvT    [T, 512]
    scoresT[k, q] = kT.T @ qT   (k on partitions -> softmax sum over k
                                 via a ones-column prepended to v)
    expT = exp(0.125 * scoresT) (no max subtraction: scores ~ N(0, 0.4))
    yT[d, q] (+ row of sums) = v_aug.T @ expT, accumulated over k tiles
    out[t, c] = yT.T-tiles @ WpT, accumulated over local j

The whole kernel is ONE software-pipelined instruction stream: the
attention inner loop is ScalarE(exp)-bound, so projection matmuls for
the NEXT t-chunk and the (deferred) output-projection matmuls are
interleaved as PE "filler" between attention k-tiles, keeping TensorE
dense for the whole span.  Softmax reciprocals are broadcast across
partitions with gpsimd.partition_broadcast (no DRAM bounce).
"""

from collections import deque
from contextlib import ExitStack

import numpy as np

import concourse.bass as bass
import concourse.tile as tile
from concourse import bacc, mybir

F32 = mybir.dt.float32
BF16 = mybir.dt.bfloat16

B, T, C, H, D = 4, 2048, 1024, 16, 64
NCORES = 8
JL = 512            # local j dims per core (8 heads * 64)
NPAIR = 4           # local head pairs
CI = C // 128       # 8 c-tiles
NT = T // 128       # 16 t/k tiles
NQC = T // 512      # 4 q chunks
VW = D + 1          # ones column + head dim

_CACHED_NC = None


def build_nc():
    nc = bacc.Bacc(None)

    xT = nc.dram_tensor("xT", [C, T], BF16, kind="ExternalInput")
    wqT = nc.dram_tensor("wqT", [C, JL], BF16, kind="ExternalInput")
    wkT = nc.dram_tensor("wkT", [C, JL], BF16, kind="ExternalInput")
    wvT = nc.dram_tensor("wvT", [C, JL], BF16, kind="ExternalInput")
    wpT = nc.dram_tensor("wpT", [JL, C], BF16, kind="ExternalInput")
    out = nc.dram_tensor("out", [T, C], BF16, kind="ExternalOutput")

    xT_r = xT.rearrange("(ci p) t -> p ci t", p=128)
    wq_r = wqT.rearrange("(ci p) j -> p ci j", p=128)
    wk_r = wkT.rearrange("(ci p) j -> p ci j", p=128)
    wv_r = wvT.rearrange("(ci p) j -> p ci j", p=128)
    wp_r = wpT.rearrange("(ji p) c -> p ji c", p=128)

    with tile.TileContext(nc) as tc, ExitStack() as ctx:
        # ---- SBUF pools --------------------------------------------------
        pm = ctx.enter_context(tc.tile_pool(name="pm", bufs=1))
        xp = ctx.enter_context(tc.tile_pool(name="xp", bufs=2))
        expp = ctx.enter_context(tc.tile_pool(name="expp", bufs=4))
        ycp = ctx.enter_context(tc.tile_pool(name="ycp", bufs=2))
        rcp = ctx.enter_context(tc.tile_pool(name="rcp", bufs=2))
        bcp = ctx.enter_context(tc.tile_pool(name="bcp", bufs=2))
        stp = ctx.enter_context(tc.tile_pool(name="stp", bufs=2))
        outp = ctx.enter_context(tc.tile_pool(name="outp", bufs=3))
        # ---- PSUM: scores ring 2x2 banks, y 2 banks, filler accs 2x1 ----
        gp = ctx.enter_context(tc.tile_pool(name="gp", bufs=2, space="PSUM"))
        yp = ctx.enter_context(tc.tile_pool(name="yp", bufs=1, space="PSUM"))
        fap = ctx.enter_context(tc.tile_pool(name="fap", bufs=2, space="PSUM"))

        wq_sb = pm.tile([128, CI, JL], BF16, tag="wq")
        wk_sb = pm.tile([128, CI, JL], BF16, tag="wk")
        wv_sb = pm.tile([128, CI, JL], BF16, tag="wv")
        wp_sb = pm.tile([128, NPAIR, C], BF16, tag="wp")
        qT_all = pm.tile([128, NPAIR, T], BF16, tag="qT_all")
        kT_all = pm.tile([128, NPAIR, T], BF16, tag="kT_all")
        yT_all = pm.tile([128, NPAIR, T], BF16, tag="yT_all")

        # v with a ones column prepended per head (softmax sums land on
        # psum partition 0, where the reciprocal runs), plus 64 pad
        # columns so every per-head lhsT can be read as [128, 128] --
        # NumWeights==128 keeps the fast weight load path.
        v_sb = pm.tile([128, NT, 8 * VW + 64], BF16, tag="v")
        v_view = v_sb[:, :, 0 : 8 * VW].rearrange("p n (h w) -> p n h w", w=VW)
        ones_col = pm.tile([128, NT, 8, 1], F32, tag="ones")
        nc.vector.memset(ones_col[:], 1.0)
        nc.vector.tensor_copy(v_view[:, :, :, 0:1], ones_col[:])
        nc.vector.memset(v_sb[:, :, 8 * VW : 8 * VW + 64], 0.0)

        # ---- input DMAs (wq/x0 interleaved so matmuls start early) ------
        xts = [None] * NQC
        xts[0] = xp.tile([128, CI, 512], BF16, tag="xt", name="xt0")
        for ci in range(CI):
            nc.sync.dma_start(wq_sb[:, ci, :], wq_r[:, ci, :])
            nc.sync.dma_start(xts[0][:, ci, :], xT_r[:, ci, 0:512])
        for ci in range(CI):
            nc.sync.dma_start(wk_sb[:, ci, :], wk_r[:, ci, :])
        for ci in range(CI):
            nc.sync.dma_start(wv_sb[:, ci, :], wv_r[:, ci, :])
        xts[1] = xp.tile([128, CI, 512], BF16, tag="xt", name="xt1")
        nc.sync.dma_start(xts[1][:], xT_r[:, :, 512:1024])
        nc.sync.dma_start(wp_sb[:], wp_r[:])

        # ---- filler machinery (PE work interleaved into attention) ------
        fgens = deque()

        def pump(n):
            done = 0
            while done < n and fgens:
                try:
                    next(fgens[0])
                    done += 1
                except StopIteration:
                    fgens.popleft()
            return done

        def run_all(gen):
            for _ in gen:
                pass

        def qk_group(xt, w_sb, pr, dst, ts):
            acc = fap.tile([128, 512], F32, tag="fa")
            for ci in range(CI):
                nc.tensor.matmul(
                    acc[:],
                    w_sb[:, ci, pr * 128 : pr * 128 + 128],
                    xt[:, ci, :],
                    start=(ci == 0),
                    stop=(ci == CI - 1),
                )
                yield
            nc.vector.tensor_copy(dst[:, pr, ts], acc[:])

        def v_group(xt, tch, tl):
            ti = tch * 4 + tl
            acc = fap.tile([128, 512], F32, tag="fa")
            for ci in range(CI):
                nc.tensor.matmul(
                    acc[:],
                    xt[:, ci, tl * 128 : tl * 128 + 128],
                    wv_sb[:, ci, :],
                    start=(ci == 0),
                    stop=(ci == CI - 1),
                )
                yield
            nc.vector.tensor_copy(
                v_view[:, ti, :, 1 : D + 1],
                acc[:].rearrange("p (h d) -> p h d", d=D),
            )

        def op_group(ti, cc):
            acc = fap.tile([128, 512], F32, tag="fa")
            tss = slice(ti * 128, ti * 128 + 128)
            cs = slice(cc * 512, cc * 512 + 512)
            for ji in range(NPAIR):
                nc.tensor.matmul(
                    acc[:],
                    yT_all[:, ji, tss],
                    wp_sb[:, ji, cs],
                    start=(ji == 0),
                    stop=(ji == NPAIR - 1),
                )
                yield
            o = outp.tile([128, 512], BF16, tag="o")
            nc.vector.tensor_copy(o[:], acc[:])
            nc.sync.dma_start(out[tss, cs], o[:])

        def queue_proj(tch):
            ts = slice(tch * 512, tch * 512 + 512)
            for pr in range(NPAIR):
                fgens.append(qk_group(xts[tch], wq_sb, pr, qT_all, ts))
                fgens.append(qk_group(xts[tch], wk_sb, pr, kT_all, ts))
            for tl in range(4):
                fgens.append(v_group(xts[tch], tch, tl))

        # ---- projection chunk 0 (PE-dense ramp) -------------------------
        ts0 = slice(0, 512)
        for pr in range(NPAIR):
            run_all(qk_group(xts[0], wq_sb, pr, qT_all, ts0))
        for pr in range(NPAIR):
            run_all(qk_group(xts[0], wk_sb, pr, kT_all, ts0))
        for tl in range(4):
            run_all(v_group(xts[0], 0, tl))

        # ---- fused attention + filler windows ---------------------------
        CREDITS = {0: 6.0, 1: 3.0, 2: 2.0, 3: 1.5}
        for qc in range(NQC):
            # prefetch the x chunk needed by the NEXT window's filler
            if qc + 2 < NQC:
                xts[qc + 2] = xp.tile([128, CI, 512], BF16, tag="xt", name=f"xt{qc+2}")
                nc.sync.dma_start(
                    xts[qc + 2][:], xT_r[:, :, (qc + 2) * 512 : (qc + 3) * 512]
                )
            if qc + 1 < NQC:
                queue_proj(qc + 1)
            else:
                for tch in range(3):
                    for ti in range(tch * 4, tch * 4 + 4):
                        for cc in range(2):
                            fgens.append(op_group(ti, cc))

            nkt = 4 * qc + 4
            qs = slice(qc * 512, qc * 512 + 512)
            credits = 0.0
            for pr in range(NPAIR):
                y = yp.tile([128, 2, 512], F32, tag="y")

                def emit_pv(kt, e, y=y, nkt=nkt, qc=qc, pr=pr):
                    dt = kt - 4 * qc
                    lo = dt * 128 if dt > 0 else 0
                    nc.tensor.matmul(
                        y[:, 0, lo:512],
                        v_sb[:, kt, 2 * pr * VW : 2 * pr * VW + 128],
                        e[:, 0, lo:512],
                        start=(kt == 0),
                        stop=(kt == nkt - 1),
                    )
                    nc.tensor.matmul(
                        y[:, 1, lo:512],
                        v_sb[:, kt, (2 * pr + 1) * VW : (2 * pr + 1) * VW + 128],
                        e[:, 1, lo:512],
                        start=(kt == 0),
                        stop=(kt == nkt - 1),
                    )

                prev = None
                for kt in range(nkt):
                    dt = kt - 4 * qc
                    xlo = dt * 128 if dt > 0 else 0
                    ks = slice(kt * 128, kt * 128 + 128)
                    qsw = slice(qc * 512 + xlo, qc * 512 + 512)
                    g = gp.tile([128, 2, 512], F32, tag="g")
                    nc.tensor.matmul(
                        g[:, 0, xlo:512],
                        kT_all[0:64, pr, ks],
                        qT_all[0:64, pr, qsw],
                        start=True,
                        stop=True,
                    )
                    nc.tensor.matmul(
                        g[:, 1, xlo:512],
                        kT_all[64:128, pr, ks],
                        qT_all[64:128, pr, qsw],
                        start=True,
                        stop=True,
                    )
                    e = expp.tile([128, 2, 512], BF16, tag="e")
                    nc.scalar.activation(
                        e[:, :, xlo:512],
                        g[:, :, xlo:512],
                        mybir.ActivationFunctionType.Exp,
                        scale=0.125,
                    )
                    if dt >= 0:
                        # zero the causal triangle (k > q) of the diagonal
                        # block on the gpsimd engine
                        bs = slice(dt * 128, dt * 128 + 128)
                        for h in (0, 1):
                            nc.gpsimd.affine_select(
                                out=e[:, h, bs],
                                in_=e[:, h, bs],
                                compare_op=mybir.AluOpType.is_ge,
                                fill=0.0,
                                base=0,
                                pattern=[[1, 128]],
                                channel_multiplier=-1,
                            )
                    credits = min(credits + CREDITS[qc] + (2.0 if kt < 2 else 0.0), 10.0)
                    credits -= pump(int(credits))
                    if prev is not None:
                        emit_pv(*prev)
                    prev = (kt, e)
                emit_pv(*prev)

                # normalize: y rows 0..64 / rowsum (sums on psum partition 0).
                # Copy out of psum first so the y banks free up for the next
                # head-pair; broadcast the reciprocals across partitions on
                # gpsimd (no DRAM bounce).
                yc = ycp.tile([D + 1, 2, 512], F32, tag="yc")
                nc.vector.tensor_copy(yc[:], y[0 : D + 1, :, :])
                rc = rcp.tile([1, 2, 512], F32, tag="rc")
                nc.vector.reciprocal_approx_fast(rc[0:1, :, :], yc[0:1, :, :])
                bc = bcp.tile([D + 1, 2, 512], F32, tag="bc")
                nc.gpsimd.partition_broadcast(bc[:], rc[0:1, :, :])
                stg = stp.tile([D + 1, 2, 512], BF16, tag="stg")
                nc.vector.tensor_mul(stg[:], yc[:], bc[:])
                nc.sync.dma_start(yT_all[0:64, pr, qs], stg[1 : D + 1, 0, :])
                nc.sync.dma_start(yT_all[64:128, pr, qs], stg[1 : D + 1, 1, :])
            pump(1 << 30)

        # ---- tail: output projection for the last t-chunk ---------------
        for ti in range(12, 16):
            for cc in range(2):
                run_all(op_group(ti, cc))

    nc.finalize()
    return nc


def _get_nc():
    global _CACHED_NC
    if _CACHED_NC is None:
        _CACHED_NC = build_nc()
    return _CACHED_NC


def kernel(x, Wq, Wk, Wv, Wp):
    import ml_dtypes
    from concourse.bass_utils import run_bass_kernel_spmd

    bf16 = ml_dtypes.bfloat16
    x = np.asarray(x, dtype=np.float32)
    Wq = np.asarray(Wq, dtype=np.float32)
    Wk = np.asarray(Wk, dtype=np.float32)
    Wv = np.asarray(Wv, dtype=np.float32)
    Wp = np.asarray(Wp, dtype=np.float32)

    nc = _get_nc()

    xT = [np.ascontiguousarray(x[b].T).astype(bf16) for b in range(B)]
    wqT, wkT, wvT, wpT = [], [], [], []
    for hh in range(2):
        js = slice(JL * hh, JL * hh + JL)
        wqT.append(np.ascontiguousarray(Wq[js, :].T.astype(bf16)))
        wkT.append(np.ascontiguousarray(Wk[js, :].T.astype(bf16)))
        wvT.append(np.ascontiguousarray(Wv[js, :].T.astype(bf16)))
        wpT.append(np.ascontiguousarray(Wp[:, js].T.astype(bf16)))

    in_maps = []
    for c in range(NCORES):
        b, hh = c // 2, c % 2
        in_maps.append(
            {
                "xT": xT[b],
                "wqT": wqT[hh],
                "wkT": wkT[hh],
                "wvT": wvT[hh],
                "wpT": wpT[hh],
            }
        )

    res = run_bass_kernel_spmd(nc, in_maps, core_ids=list(range(NCORES)))

    out = np.empty((B, T, C), dtype=np.float32)
    for b in range(B):
        out[b] = res.results[2 * b]["out"].astype(np.float32) + res.results[
            2 * b + 1
        ]["out"].astype(np.float32)
    return out


# revision 8
# speedup vs baseline: 1.2568x; 1.0563x over previous
"""Causal self-attention Bass kernel for 8 TRN2 NeuronCores.

Problem: B=4, T=2048, C=1024, H=16 heads, head_dim=64, fp32.
    q = x @ Wq.T ; k = x @ Wk.T ; v = x @ Wv.T          (per head)
    att = softmax(mask(q k^T / 8))
    y = att @ v ; out = y @ Wp.T

Sharding (8 cores): 4-way data parallel over batch x 2-way tensor
parallel over heads. Core c handles batch c//2 and heads 8*(c%2)..+8.
Wq/Wk/Wv column-parallel, Wp row-parallel; the partial outputs of the
two head-halves of each batch are summed on the host (the "all-reduce"
of row-parallel Wp).

Device dataflow (all bf16 operands so every weight load takes the fast
path and DMA traffic halves; psum accumulation stays fp32):
    xT [C, T] (host-pretransposed) ->
    qT/kT = WqT.T-slices @ xT   [512, T]  (pairs of heads on partitions)
    v     = xT.T-tiles @ WvT    [T, 512]
    scoresT[k, q] = kT.T @ qT   (k on partitions -> softmax sum over k
                                 via a ones-column prepended to v)
    expT = exp(0.125 * scoresT) (no max subtraction: scores ~ N(0, 0.4))
    yT[d, q] (+ row of sums) = v_aug.T @ expT, accumulated over k tiles
    out[t, c] = yT.T-tiles @ WpT, accumulated over local j

The whole kernel is ONE software-pipelined instruction stream: the
attention inner loop is ScalarE(exp)-bound, so projection matmuls for
the NEXT t-chunk and the (deferred) output-projection matmuls are
interleaved as PE "filler" between attention k-tiles, keeping TensorE
dense for the whole span.  Softmax reciprocals are broadcast across
partitions with gpsimd.partition_broadcast (no DRAM bounce).
"""

from collections import deque
from contextlib import ExitStack

import numpy as np

import concourse.bass as bass
import concourse.tile as tile
from concourse import bacc, mybir

F32 = mybir.dt.float32
BF16 = mybir.dt.bfloat16

B, T, C, H, D = 4, 2048, 1024, 16, 64
NCORES = 8
JL = 512            # local j dims per core (8 heads * 64)
NPAIR = 4           # local head pairs
CI = C // 128       # 8 c-tiles
NT = T // 128       # 16 t/k tiles
NQC = T // 512      # 4 q chunks
VW = D + 1          # ones column + head dim

_CACHED_NC = None


def build_nc():
    nc = bacc.Bacc(None)

    xT = nc.dram_tensor("xT", [C, T], BF16, kind="ExternalInput")
    wqT = nc.dram_tensor("wqT", [C, JL], BF16, kind="ExternalInput")
    wkT = nc.dram_tensor("wkT", [C, JL], BF16, kind="ExternalInput")
    wvT = nc.dram_tensor("wvT", [C, JL], BF16, kind="ExternalInput")
    wpT = nc.dram_tensor("wpT", [JL, C], BF16, kind="ExternalInput")
    out = nc.dram_tensor("out", [T, C], BF16, kind="ExternalOutput")

    xT_r = xT.rearrange("(ci p) t -> p ci t", p=128)
    wq_r = wqT.rearrange("(ci p) j -> p ci j", p=128)
    wk_r = wkT.rearrange("(ci p) j -> p ci j", p=128)
    wv_r = wvT.rearrange("(ci p) j -> p ci j", p=128)
    wp_r = wpT.rearrange("(ji p) c -> p ji c", p=128)

    with tile.TileContext(nc) as tc, ExitStack() as ctx:
        # ---- SBUF pools --------------------------------------------------
        pm = ctx.enter_context(tc.tile_pool(name="pm", bufs=1))
        xp = ctx.enter_context(tc.tile_pool(name="xp", bufs=2))
        expp = ctx.enter_context(tc.tile_pool(name="expp", bufs=4))
        ycp = ctx.enter_context(tc.tile_pool(name="ycp", bufs=2))
        rcp = ctx.enter_context(tc.tile_pool(name="rcp", bufs=2))
        bcp = ctx.enter_context(tc.tile_pool(name="bcp", bufs=2))
        stp = ctx.enter_context(tc.tile_pool(name="stp", bufs=2))
        outp = ctx.enter_context(tc.tile_pool(name="outp", bufs=3))
        # ---- PSUM: scores ring 2x2 banks, y 2 banks, filler accs 2x1 ----
        gp = ctx.enter_context(tc.tile_pool(name="gp", bufs=2, space="PSUM"))
        yp = ctx.enter_context(tc.tile_pool(name="yp", bufs=1, space="PSUM"))
        fap = ctx.enter_context(tc.tile_pool(name="fap", bufs=2, space="PSUM"))

        wq_sb = pm.tile([128, CI, JL], BF16, tag="wq")
        wk_sb = pm.tile([128, CI, JL], BF16, tag="wk")
        wv_sb = pm.tile([128, CI, JL], BF16, tag="wv")
        wp_sb = pm.tile([128, NPAIR, C], BF16, tag="wp")
        qT_all = pm.tile([128, NPAIR, T], BF16, tag="qT_all")
        kT_all = pm.tile([128, NPAIR, T], BF16, tag="kT_all")
        yT_all = pm.tile([128, NPAIR, T], BF16, tag="yT_all")

        # v with a ones column prepended per head (softmax sums land on
        # psum partition 0, where the reciprocal runs), plus 64 pad
        # columns so every per-head lhsT can be read as [128, 128] --
        # NumWeights==128 keeps the fast weight load path.
        v_sb = pm.tile([128, NT, 8 * VW + 64], BF16, tag="v")
        v_view = v_sb[:, :, 0 : 8 * VW].rearrange("p n (h w) -> p n h w", w=VW)
        ones_col = pm.tile([128, NT, 8, 1], F32, tag="ones")
        nc.vector.memset(ones_col[:], 1.0)
        nc.vector.tensor_copy(v_view[:, :, :, 0:1], ones_col[:])
        nc.vector.memset(v_sb[:, :, 8 * VW : 8 * VW + 64], 0.0)

        # ---- input DMAs (wq/x0 interleaved 2-ci granules: few issues, ----
        # ---- but the first projection matmuls can still start early) ----
        xts = [None] * NQC
        xts[0] = xp.tile([128, CI, 512], BF16, tag="xt", name="xt0")
        for c2 in range(0, CI, 2):
            nc.sync.dma_start(wq_sb[:, c2 : c2 + 2, :], wq_r[:, c2 : c2 + 2, :])
            nc.sync.dma_start(
                xts[0][:, c2 : c2 + 2, :], xT_r[:, c2 : c2 + 2, 0:512]
            )
        for c4 in range(0, CI, 4):
            nc.sync.dma_start(wk_sb[:, c4 : c4 + 4, :], wk_r[:, c4 : c4 + 4, :])
        for c4 in range(0, CI, 4):
            nc.sync.dma_start(wv_sb[:, c4 : c4 + 4, :], wv_r[:, c4 : c4 + 4, :])
        xts[1] = xp.tile([128, CI, 512], BF16, tag="xt", name="xt1")
        nc.sync.dma_start(xts[1][:], xT_r[:, :, 512:1024])
        nc.sync.dma_start(wp_sb[:], wp_r[:])

        # ---- filler machinery (PE work interleaved into attention) ------
        fgens = deque()

        def pump(n):
            done = 0
            while done < n and fgens:
                try:
                    next(fgens[0])
                    done += 1
                except StopIteration:
                    fgens.popleft()
            return done

        def run_all(gen):
            for _ in gen:
                pass

        def qk_group(xt, w_sb, pr, dst, ts):
            acc = fap.tile([128, 512], F32, tag="fa")
            for ci in range(CI):
                nc.tensor.matmul(
                    acc[:],
                    w_sb[:, ci, pr * 128 : pr * 128 + 128],
                    xt[:, ci, :],
                    start=(ci == 0),
                    stop=(ci == CI - 1),
                )
                yield
            nc.vector.tensor_copy(dst[:, pr, ts], acc[:])

        def v_group(xt, tch, tl):
            ti = tch * 4 + tl
            acc = fap.tile([128, 512], F32, tag="fa")
            for ci in range(CI):
                nc.tensor.matmul(
                    acc[:],
                    xt[:, ci, tl * 128 : tl * 128 + 128],
                    wv_sb[:, ci, :],
                    start=(ci == 0),
                    stop=(ci == CI - 1),
                )
                yield
            nc.vector.tensor_copy(
                v_view[:, ti, :, 1 : D + 1],
                acc[:].rearrange("p (h d) -> p h d", d=D),
            )

        def op_group(ti, cc):
            acc = fap.tile([128, 512], F32, tag="fa")
            tss = slice(ti * 128, ti * 128 + 128)
            cs = slice(cc * 512, cc * 512 + 512)
            for ji in range(NPAIR):
                nc.tensor.matmul(
                    acc[:],
                    yT_all[:, ji, tss],
                    wp_sb[:, ji, cs],
                    start=(ji == 0),
                    stop=(ji == NPAIR - 1),
                )
                yield
            o = outp.tile([128, 512], BF16, tag="o")
            nc.vector.tensor_copy(o[:], acc[:])
            nc.sync.dma_start(out[tss, cs], o[:])

        def queue_proj(tch):
            ts = slice(tch * 512, tch * 512 + 512)
            for pr in range(NPAIR):
                fgens.append(qk_group(xts[tch], wq_sb, pr, qT_all, ts))
                fgens.append(qk_group(xts[tch], wk_sb, pr, kT_all, ts))
            for tl in range(4):
                fgens.append(v_group(xts[tch], tch, tl))

        # ---- projection chunk 0 (PE-dense ramp) -------------------------
        ts0 = slice(0, 512)
        for pr in range(NPAIR):
            run_all(qk_group(xts[0], wq_sb, pr, qT_all, ts0))
        for pr in range(NPAIR):
            run_all(qk_group(xts[0], wk_sb, pr, kT_all, ts0))
        for tl in range(4):
            run_all(v_group(xts[0], 0, tl))

        # ---- fused attention + filler windows ---------------------------
        CREDITS = {0: 6.0, 1: 3.0, 2: 2.0, 3: 1.5}
        pend_norm = deque()
        for qc in range(NQC):
            # prefetch the x chunk needed by the NEXT window's filler
            if qc + 2 < NQC:
                xts[qc + 2] = xp.tile([128, CI, 512], BF16, tag="xt", name=f"xt{qc+2}")
                nc.sync.dma_start(
                    xts[qc + 2][:], xT_r[:, :, (qc + 2) * 512 : (qc + 3) * 512]
                )
            if qc + 1 < NQC:
                queue_proj(qc + 1)
            else:
                for tch in range(3):
                    for ti in range(tch * 4, tch * 4 + 4):
                        for cc in range(2):
                            fgens.append(op_group(ti, cc))

            nkt = 4 * qc + 4
            qs = slice(qc * 512, qc * 512 + 512)
            credits = 0.0
            for pr in range(NPAIR):
                y = yp.tile([128, 2, 512], F32, tag="y")
                last = qc == NQC - 1 and pr == NPAIR - 1

                def emit_pv(kt, e, y=y, nkt=nkt, qc=qc, pr=pr):
                    dt = kt - 4 * qc
                    lo = dt * 128 if dt > 0 else 0
                    nc.tensor.matmul(
                        y[:, 0, lo:512],
                        v_sb[:, kt, 2 * pr * VW : 2 * pr * VW + 128],
                        e[:, 0, lo:512],
                        start=(kt == 0),
                        stop=(kt == nkt - 1),
                    )
                    nc.tensor.matmul(
                        y[:, 1, lo:512],
                        v_sb[:, kt, (2 * pr + 1) * VW : (2 * pr + 1) * VW + 128],
                        e[:, 1, lo:512],
                        start=(kt == 0),
                        stop=(kt == nkt - 1),
                    )

                pend_pv = deque()
                for kt in range(nkt):
                    dt = kt - 4 * qc
                    xlo = dt * 128 if dt > 0 else 0
                    ks = slice(kt * 128, kt * 128 + 128)
                    qsw = slice(qc * 512 + xlo, qc * 512 + 512)
                    g = gp.tile([128, 2, 512], F32, tag="g")
                    nc.tensor.matmul(
                        g[:, 0, xlo:512],
                        kT_all[0:64, pr, ks],
                        qT_all[0:64, pr, qsw],
                        start=True,
                        stop=True,
                        tile_position=(0, 0),
                    )
                    nc.tensor.matmul(
                        g[:, 1, xlo:512],
                        kT_all[64:128, pr, ks],
                        qT_all[64:128, pr, qsw],
                        start=True,
                        stop=True,
                        tile_position=(64, 0),
                    )
                    e = expp.tile([128, 2, 512], BF16, tag="e")
                    nc.scalar.activation(
                        e[:, :, xlo:512],
                        g[:, :, xlo:512],
                        mybir.ActivationFunctionType.Exp,
                        scale=0.125,
                    )
                    if dt >= 0:
                        # zero the causal triangle (k > q) of the diagonal
                        # block on the gpsimd engine
                        bs = slice(dt * 128, dt * 128 + 128)
                        for h in (0, 1):
                            nc.gpsimd.affine_select(
                                out=e[:, h, bs],
                                in_=e[:, h, bs],
                                compare_op=mybir.AluOpType.is_ge,
                                fill=0.0,
                                base=0,
                                pattern=[[1, 128]],
                                channel_multiplier=-1,
                            )
                    if kt == 2 and pend_norm:
                        pend_norm.popleft()()
                    credits = min(credits + CREDITS[qc] + (2.0 if kt < 2 else 0.0), 10.0)
                    credits -= pump(int(credits))
                    pend_pv.append((kt, e))
                    # PV runs two k-tiles behind the scores/exp stream so it
                    # never waits on the activation
                    if len(pend_pv) > 2:
                        emit_pv(*pend_pv.popleft())
                while pend_pv:
                    emit_pv(*pend_pv.popleft())

                # normalize: y rows 0..64 / rowsum (sums on psum partition 0).
                # Copy out of psum promptly so the y banks free up for the
                # next head-pair; defer the rest of the chain (reciprocal,
                # gpsimd partition-broadcast, scale, yT stores) into the next
                # head-pair's k-loop so it never blocks the gpsimd masks.
                if not last:
                    yc = ycp.tile([D + 1, 2, 512], F32, tag="yc")
                    nc.vector.tensor_copy(yc[:], y[0 : D + 1, :, :])

                    def norm_chain(yc=yc, pr=pr, qs=qs):
                        rc = rcp.tile([1, 2, 512], F32, tag="rc")
                        nc.vector.reciprocal_approx_fast(rc[0:1, :, :], yc[0:1, :, :])
                        bc = bcp.tile([D + 1, 2, 512], F32, tag="bc")
                        nc.gpsimd.partition_broadcast(bc[:], rc[0:1, :, :])
                        stg = stp.tile([D + 1, 2, 512], BF16, tag="stg")
                        nc.vector.tensor_mul(stg[:], yc[:], bc[:])
                        nc.sync.dma_start(yT_all[0:64, pr, qs], stg[1 : D + 1, 0, :])
                        nc.sync.dma_start(yT_all[64:128, pr, qs], stg[1 : D + 1, 1, :])

                    pend_norm.append(norm_chain)
                else:
                    # tail fast path: minimum-latency chain straight out of
                    # psum — reciprocal from psum row 0, gpsimd broadcast,
                    # scale from psum, and split the yT stores across two
                    # queues.
                    rc = rcp.tile([1, 2, 512], F32, tag="rc")
                    nc.vector.reciprocal_approx_fast(rc[0:1, :, :], y[0:1, :, :])
                    bc = bcp.tile([D + 1, 2, 512], F32, tag="bc")
                    nc.gpsimd.partition_broadcast(bc[:], rc[0:1, :, :])
                    stg = stp.tile([D + 1, 2, 512], BF16, tag="stg")
                    nc.vector.tensor_mul(stg[:], y[0 : D + 1, :, :], bc[:])
                    nc.sync.dma_start(yT_all[0:64, pr, qs], stg[1 : D + 1, 0, :])
                    nc.scalar.dma_start(yT_all[64:128, pr, qs], stg[1 : D + 1, 1, :])
            pump(1 << 30)
        while pend_norm:
            pend_norm.popleft()()

        # ---- tail: output projection for the last t-chunk ---------------
        for ti in range(12, 16):
            for cc in range(2):
                run_all(op_group(ti, cc))

    nc.finalize()
    return nc


def _get_nc():
    global _CACHED_NC
    if _CACHED_NC is None:
        _CACHED_NC = build_nc()
    return _CACHED_NC


def kernel(x, Wq, Wk, Wv, Wp):
    import ml_dtypes
    from concourse.bass_utils import run_bass_kernel_spmd

    bf16 = ml_dtypes.bfloat16
    x = np.asarray(x, dtype=np.float32)
    Wq = np.asarray(Wq, dtype=np.float32)
    Wk = np.asarray(Wk, dtype=np.float32)
    Wv = np.asarray(Wv, dtype=np.float32)
    Wp = np.asarray(Wp, dtype=np.float32)

    nc = _get_nc()

    xT = [np.ascontiguousarray(x[b].T).astype(bf16) for b in range(B)]
    wqT, wkT, wvT, wpT = [], [], [], []
    for hh in range(2):
        js = slice(JL * hh, JL * hh + JL)
        wqT.append(np.ascontiguousarray(Wq[js, :].T.astype(bf16)))
        wkT.append(np.ascontiguousarray(Wk[js, :].T.astype(bf16)))
        wvT.append(np.ascontiguousarray(Wv[js, :].T.astype(bf16)))
        wpT.append(np.ascontiguousarray(Wp[:, js].T.astype(bf16)))

    in_maps = []
    for c in range(NCORES):
        b, hh = c // 2, c % 2
        in_maps.append(
            {
                "xT": xT[b],
                "wqT": wqT[hh],
                "wkT": wkT[hh],
                "wvT": wvT[hh],
                "wpT": wpT[hh],
            }
        )

    res = run_bass_kernel_spmd(nc, in_maps, core_ids=list(range(NCORES)))

    out = np.empty((B, T, C), dtype=np.float32)
    for b in range(B):
        out[b] = res.results[2 * b]["out"].astype(np.float32) + res.results[
            2 * b + 1
        ]["out"].astype(np.float32)
    return out
